# revision 13
# baseline (speedup 1.0000x reference)
"""Trainium2 Bass kernel for nn_DecoderRNN (GRU decoder, 140 sequential steps).

Strategy (data-parallel, per sharding hint):
  - B=512 sharded 8 ways -> 64 batch rows per core; weights replicated.
  - All tensors on-chip are feature-major: a [F, B] tensor is stored as
    F/128 chunks of [128 partitions, 64 batch] side by side in the free dim.
  - Matmuls: out[M,N] = lhsT.T @ rhs with lhsT = weight tile [K=128, M=128]
    (stationary, bf16 -> fast weight load), rhs = activation chunk [128, 64].
  - GRU gate trick: r and z need only (Wih e + Whh h + b); both matmul
    groups accumulate into the same PSUM region, biases folded into the
    ACT sigmoid's per-partition bias. n = tanh(inn + b_in + r*(hn + b_hn))
    via one fused scalar_tensor_tensor + one tensor_tensor + ACT tanh.
  - The final regression projection (reg_W @ token) is done on the fly:
    one encoder token + the fresh decoder token per RNN step (2+2 matmuls),
    output DMA'd per token. No big dec buffer, no post-loop phase.
  - Everything bf16 except PSUM (fp32) and the f32 output; validated in
    numpy simulation at absmax relative error ~2e-3 vs the f32 reference.
"""

import numpy as np
import ml_dtypes

B, T_ENC, E, H, O, PRED_LEN = 512, 140, 256, 512, 64, 140
NCORES = 8
BC = B // NCORES           # 64 batch rows per core
T_ALL = T_ENC + PRED_LEN   # 280

bf16 = ml_dtypes.bfloat16

# M-chunk order for the r/z part of the gates: interleave r and z 128-chunks
# so chunk pair c = (r_c, z_c) lands in one PSUM bank and the whole per-
# feature-chunk gate pipeline works on adjacent slices.
RZ_ORDER = [0, 4, 1, 5, 2, 6, 3, 7]  # of the 12 M-chunks of [r z n] layout


def _pack_tiles(wT, n_k, n_m, m_order=None):
    """Pack a [K, M] (pre-transposed) weight into [128, n_m*n_k*128] bf16:
    tile (mi, k) at cols (mi*n_k + k)*128."""
    K, M = wT.shape
    assert K == n_k * 128 and M == n_m * 128
    t = wT.reshape(n_k, 128, n_m, 128).transpose(2, 0, 1, 3)  # [mc, kc, 128, 128]
    if m_order is not None:
        t = t[m_order]
    # -> [128(part), mc, kc, 128]
    t = t.transpose(2, 0, 1, 3).reshape(128, -1)
    return np.ascontiguousarray(t.astype(bf16))


def _feat_major(x, n_chunks):
    """[B, F] -> [128, n_chunks*B] feature-major chunk layout."""
    b, f = x.shape
    assert f == n_chunks * 128
    t = x.reshape(b, n_chunks, 128).transpose(2, 1, 0).reshape(128, n_chunks * b)
    return np.ascontiguousarray(t)


def build_program(nsteps=PRED_LEN, t_enc=T_ENC, lowering=True):
    """Build the Bass program (per-core SPMD). Returns nc.

    lowering=True -> Bacc + target_bir_lowering (full walrus pipeline; the
    raw-BIR codegen path rejects Tile's multi-wait instructions on this
    toolchain). lowering=False -> plain Bass for CoreSim smoke tests.
    """
    import concourse.bass as bass
    import concourse.tile as tile
    from concourse import bacc, mybir

    AF = mybir.ActivationFunctionType
    OP = mybir.AluOpType
    f32 = mybir.dt.float32
    bf = mybir.dt.bfloat16

    t_all = t_enc + nsteps

    if lowering:
        nc = bacc.Bacc("TRN2", target_bir_lowering=True, debug=False)
    else:
        nc = bass.Bass("TRN2", target_bir_lowering=False, debug=False)

    # ---- DRAM I/O ----
    encT_d = nc.dram_tensor("encT", [128, t_enc * 128], bf, kind="ExternalInput").ap()
    h0_d = nc.dram_tensor("h0T", [128, 4 * BC], bf, kind="ExternalInput").ap()
    wih_d = nc.dram_tensor("wihT", [128, 48 * 128], bf, kind="ExternalInput").ap()
    whh_d = nc.dram_tensor("whhT", [128, 48 * 128], bf, kind="ExternalInput").ap()
    emb_d = nc.dram_tensor("embT", [128, 8 * 128], bf, kind="ExternalInput").ap()
    outw_d = nc.dram_tensor("outwT", [128, 8 * 128], bf, kind="ExternalInput").ap()
    regw_d = nc.dram_tensor("regwT", [128, 2 * O], bf, kind="ExternalInput").ap()
    brz_d = nc.dram_tensor("b_rz", [128, 512], f32, kind="ExternalInput").ap()
    bhn_d = nc.dram_tensor("b_hn", [128, 4], f32, kind="ExternalInput").ap()
    bin_d = nc.dram_tensor("b_in", [128, 4], f32, kind="ExternalInput").ap()
    be_d = nc.dram_tensor("b_e", [128, 4], f32, kind="ExternalInput").ap()
    bo_d = nc.dram_tensor("b_o", [128, 128], f32, kind="ExternalInput").ap()
    breg_d = nc.dram_tensor("b_reg", [BC, O], f32, kind="ExternalInput").ap()
    y_d = nc.dram_tensor("y", [BC, t_all, O], f32, kind="ExternalOutput").ap()

    with tile.TileContext(nc) as tc:
        import contextlib
        with contextlib.ExitStack() as ctx:
            consts = ctx.enter_context(tc.tile_pool(name="consts", bufs=1))
            temps = ctx.enter_context(tc.tile_pool(name="temps", bufs=2))
            ytmp = ctx.enter_context(tc.tile_pool(name="ytmp", bufs=3))
            psum = ctx.enter_context(tc.tile_pool(name="psum", bufs=1, space="PSUM"))

            # ---- ACT table warmup ----
            # walrus inserts the activation-table load before the first
            # ACTIVATE of the set; that extra sync blows the per-instruction
            # wait-slot budget if it lands on an instruction that already
            # has 2 waits. Pin the load to dependency-light dummy ops.
            # Relu/Sigmoid/Tanh/Identity all live in `sigmoid_and_others`.
            wt = consts.tile([128, 8], f32, tag="wtbl", name="wtbl")
            nc.vector.memset(wt[:, 0:4], 0.0)
            nc.scalar.activation(wt[:, 4:5], wt[:, 0:1], AF.Relu)
            nc.scalar.activation(wt[:, 5:6], wt[:, 1:2], AF.Sigmoid)
            nc.scalar.activation(wt[:, 6:7], wt[:, 2:3], AF.Tanh)

            # ---- load constants into SBUF ----
            wih_sb = consts.tile([128, 48 * 128], bf, tag="wih")
            whh_sb = consts.tile([128, 48 * 128], bf, tag="whh")
            emb_sb = consts.tile([128, 8 * 128], bf, tag="emb")
            outw_sb = consts.tile([128, 8 * 128], bf, tag="outw")
            regw_sb = consts.tile([128, 2 * O], bf, tag="regw")
            brz_sb = consts.tile([128, 512], f32, tag="brz")
            bhn_sb = consts.tile([128, 4], f32, tag="bhn")
            bin_sb = consts.tile([128, 4], f32, tag="bin")
            be_sb = consts.tile([128, 4], f32, tag="be")
            bo_sb = consts.tile([128, 128], f32, tag="bo")
            breg_sb = consts.tile([BC, O], f32, tag="breg")
            encT_sb = consts.tile([128, t_enc * 128], bf, tag="encT")

            nc.sync.dma_start(out=emb_sb, in_=emb_d)
            nc.sync.dma_start(out=whh_sb, in_=whh_d)
            nc.sync.dma_start(out=wih_sb, in_=wih_d)
            nc.sync.dma_start(out=outw_sb, in_=outw_d)
            nc.sync.dma_start(out=regw_sb, in_=regw_d)
            for sb, d in ((brz_sb, brz_d), (bhn_sb, bhn_d), (bin_sb, bin_d),
                          (be_sb, be_d), (bo_sb, bo_d), (breg_sb, breg_d)):
                nc.sync.dma_start(out=sb, in_=d)
            # x0 block (last encoder token) first so step 0 can start early
            lastblk = slice((t_enc - 1) * 128, t_enc * 128)
            nc.sync.dma_start(out=encT_sb[:, lastblk], in_=encT_d[:, lastblk])
            # rest of encT in 4 chunks
            nsplit = 4
            per = (t_enc - 1) // nsplit + 1
            for i in range(nsplit):
                lo = i * per
                hi = min((i + 1) * per, t_enc - 1)
                if lo >= hi:
                    continue
                nc.sync.dma_start(out=encT_sb[:, lo * 128:hi * 128],
                                  in_=encT_d[:, lo * 128:hi * 128])

            # ---- persistent state: h ping-pong (4 chunk tiles x 2) ----
            h_pp = [[consts.tile([128, BC], bf, tag=f"h{s}_{c}", name=f"h{s}_{c}")
                     for c in range(4)] for s in range(2)]
            x_pp = [consts.tile([128, 2 * BC], bf, tag=f"x{s}", name=f"x{s}")
                    for s in range(2)]
            for c in range(4):
                nc.sync.dma_start(out=h_pp[0][c], in_=h0_d[:, c * BC:(c + 1) * BC])

            def wtile(sb, mi, k, n_k):
                j = (mi * n_k + k) * 128
                return sb[:, j:j + 128]

            for t in range(nsteps):
                x_cur = encT_sb[:, lastblk] if t == 0 else x_pp[t % 2]
                x_nx = x_pp[(t + 1) % 2]
                hc = h_pp[t % 2]
                hnx = h_pp[(t + 1) % 2]

                ps_eo = psum.tile([128, 384], f32, tag="eo")    # e [0:256) out [256:384)
                ps_hn = psum.tile([128, 256], f32, tag="hn")
                ps_inn = psum.tile([128, 256], f32, tag="inn")
                ps_rz = [psum.tile([128, 128], f32, tag=f"rz{c}", name=f"ps_rz{c}")
                         for c in range(4)]
                ps_y = psum.tile([BC, 2 * O], f32, tag="y")     # enc [0:64) dec [64:128)

                # 1) hn = Whh_n @ h (h-only work first: covers the wait for x)
                for m in range(4):
                    for k in range(4):
                        nc.tensor.matmul(ps_hn[:, m * BC:(m + 1) * BC],
                                         wtile(whh_sb, 8 + m, k, 4), hc[k],
                                         start=(k == 0), stop=(k == 3))
                # 2) encoder-token projection for token t (fills PE gaps)
                if t < t_enc:
                    for k in range(2):
                        nc.tensor.matmul(ps_y[:, 0:O],
                                         encT_sb[:, t * 128 + k * BC: t * 128 + (k + 1) * BC],
                                         regw_sb[:, k * O:(k + 1) * O],
                                         start=(k == 0), stop=(k == 1))
                # 3) embedding: e_psum[m] = sum_k embT[m,k] @ x[k]
                for m in range(4):
                    for k in range(2):
                        nc.tensor.matmul(ps_eo[:, m * BC:(m + 1) * BC],
                                         wtile(emb_sb, m, k, 2),
                                         x_cur[:, k * BC:(k + 1) * BC],
                                         start=(k == 0), stop=(k == 1))
                # 4) e = relu(e_psum + b_e)  (bf16, feeds ih matmuls)
                e = temps.tile([128, 256], bf, tag="e")
                for m in range(4):
                    nc.scalar.activation(e[:, m * BC:(m + 1) * BC],
                                         ps_eo[:, m * BC:(m + 1) * BC],
                                         AF.Relu, bias=be_sb[:, m:m + 1])
                # 5) inn = Wih_n @ e (m-chunks 8..11 of wih)
                for m in range(4):
                    for k in range(4):
                        nc.tensor.matmul(ps_inn[:, m * BC:(m + 1) * BC],
                                         wtile(wih_sb, 8 + m, k, 4),
                                         e[:, k * BC:(k + 1) * BC],
                                         start=(k == 0), stop=(k == 3))
                # 6) rz accumulation: per feature-chunk c, bank rz[c] holds
                #    [r_c | z_c]; each is Whh part + Wih part (8 matmuls)
                for c in range(4):
                    for half in range(2):     # 0 -> r_c (M-chunk 2c), 1 -> z_c
                        mi = 2 * c + half
                        dst = ps_rz[c][:, half * BC:(half + 1) * BC]
                        for k in range(4):
                            nc.tensor.matmul(dst, wtile(whh_sb, mi, k, 4), hc[k],
                                             start=(k == 0), stop=False)
                        for k in range(4):
                            nc.tensor.matmul(dst, wtile(wih_sb, mi, k, 4),
                                             e[:, k * BC:(k + 1) * BC],
                                             start=False, stop=(k == 3))
                # 7) gates per chunk, staggered so h chunks stream out early.
                #    chain: rz+=bias (DVE) -> sigmoid[r|z] (ACT) -> t3 -> t4
                #    -> tanh -> h' = (1-z)*n + z*h  with zh/omz precomputed
                #    on GpSimd during the tanh window.
                rz_s = temps.tile([128, 512], bf, tag="rz_s")
                t3 = temps.tile([128, 256], bf, tag="t3")
                t4 = temps.tile([128, 256], bf, tag="t4")
                n_t = temps.tile([128, 256], bf, tag="n")
                zh = temps.tile([128, 256], bf, tag="zh")
                omz = temps.tile([128, 256], bf, tag="omz")
                t6 = temps.tile([128, 256], bf, tag="t6")
                for c in range(4):
                    cs = slice(c * BC, (c + 1) * BC)
                    bank = slice(c * 128, (c + 1) * 128)
                    r_sl = rz_s[:, c * 128: c * 128 + BC]
                    z_sl = rz_s[:, c * 128 + BC: (c + 1) * 128]
                    nc.vector.tensor_tensor(ps_rz[c], ps_rz[c], brz_sb[:, bank], OP.add)
                    nc.scalar.activation(rz_s[:, bank], ps_rz[c], AF.Sigmoid)
                    nc.gpsimd.tensor_tensor(zh[:, cs], z_sl, hc[c], OP.mult)
                    nc.gpsimd.tensor_scalar(omz[:, cs], z_sl, -1.0, 1.0,
                                            OP.mult, OP.add)
                    # t3 = (hn + b_hn) * r
                    nc.vector.scalar_tensor_tensor(t3[:, cs], ps_hn[:, cs],
                                                   bhn_sb[:, c:c + 1], r_sl,
                                                   OP.add, OP.mult)
                    nc.vector.tensor_tensor(t4[:, cs], t3[:, cs], ps_inn[:, cs], OP.add)
                    nc.scalar.activation(n_t[:, cs], t4[:, cs], AF.Tanh,
                                         bias=bin_sb[:, c:c + 1])
                    nc.vector.tensor_tensor(t6[:, cs], n_t[:, cs], omz[:, cs], OP.mult)
                    nc.vector.tensor_tensor(hnx[c], t6[:, cs], zh[:, cs], OP.add)
                # 8) out = outW @ h'  -> x_next
                for m in range(2):
                    for k in range(4):
                        nc.tensor.matmul(ps_eo[:, 256 + m * BC:256 + (m + 1) * BC],
                                         wtile(outw_sb, m, k, 4), hnx[k],
                                         start=(k == 0), stop=(k == 3))
                nc.vector.tensor_tensor(x_nx, ps_eo[:, 256:384], bo_sb, OP.add)
                # 9) decoder-token projection: y_dec[t] = regW @ x_next + b
                for k in range(2):
                    nc.tensor.matmul(ps_y[:, O:2 * O],
                                     x_nx[:, k * BC:(k + 1) * BC],
                                     regw_sb[:, k * O:(k + 1) * O],
                                     start=(k == 0), stop=(k == 1))
                # 10) stage + bias + DMA out
                if t < t_enc:
                    y_enc = ytmp.tile([BC, O], f32, tag="yenc")
                    nc.vector.tensor_tensor(y_enc, ps_y[:, 0:O], breg_sb, OP.add)
                    nc.sync.dma_start(out=y_d[:, t, :], in_=y_enc)
                y_dec = ytmp.tile([BC, O], f32, tag="ydec")
                nc.vector.tensor_tensor(y_dec, ps_y[:, O:2 * O], breg_sb, OP.add)
                nc.sync.dma_start(out=y_d[:, t_enc + t, :], in_=y_dec)

            # leftover encoder tokens if nsteps < t_enc (smoke tests only)
            for t in range(nsteps, t_enc):
                ps_y2 = psum.tile([BC, 2 * O], f32, tag="y")
                for k in range(2):
                    nc.tensor.matmul(ps_y2[:, 0:O],
                                     encT_sb[:, t * 128 + k * BC: t * 128 + (k + 1) * BC],
                                     regw_sb[:, k * O:(k + 1) * O],
                                     start=(k == 0), stop=(k == 1))
                y_enc = ytmp.tile([BC, O], f32, tag="yenc")
                nc.vector.tensor_tensor(y_enc, ps_y2[:, 0:O], breg_sb, OP.add)
                nc.sync.dma_start(out=y_d[:, t, :], in_=y_enc)

    if lowering:
        nc.finalize()
    return nc


def prep_inputs(encoder_outputs, encoder_hidden, emb_W, emb_b, w_ih, w_hh,
                b_ih, b_hh, out_W, out_b, reg_W, reg_b, nsteps=PRED_LEN,
                t_enc=T_ENC):
    """Host-side packing. Returns (shared input dict, per-core input dicts)."""
    f32 = np.float32
    emb_W, emb_b, w_ih, w_hh, b_ih, b_hh, out_W, out_b, reg_W, reg_b = (
        np.asarray(a, f32) for a in
        (emb_W, emb_b, w_ih, w_hh, b_ih, b_hh, out_W, out_b, reg_W, reg_b))

    shared = {
        "wihT": _pack_tiles(w_ih.T, 4, 12, RZ_ORDER + [8, 9, 10, 11]),
        "whhT": _pack_tiles(w_hh.T, 4, 12, RZ_ORDER + [8, 9, 10, 11]),
        "embT": _pack_tiles(emb_W.T, 2, 4),
        "outwT": _pack_tiles(out_W.T, 4, 2),
        "regwT": np.ascontiguousarray(
            reg_W.T.reshape(2, 128, O).transpose(1, 0, 2).reshape(128, 2 * O)
            .astype(bf16)),
        # b_rz broadcast: [128, 512], bank c = [r_c bias | z_c bias] each
        # replicated across the 64 batch columns
        "b_rz": np.ascontiguousarray(
            np.broadcast_to(
                (b_ih[:2 * H] + b_hh[:2 * H]).reshape(8, 128)[RZ_ORDER]
                .transpose(1, 0)[:, :, None], (128, 8, BC)).reshape(128, 512)
            .astype(f32)),
        "b_hn": np.ascontiguousarray(b_hh[2 * H:].reshape(4, 128).T.astype(f32)),
        "b_in": np.ascontiguousarray(b_ih[2 * H:].reshape(4, 128).T.astype(f32)),
        "b_e": np.ascontiguousarray(emb_b.reshape(4, 128).T.astype(f32)),
        # b_o broadcast: [128, 128], chunk m cols = out_b[m*128+p]
        "b_o": np.ascontiguousarray(
            np.broadcast_to(out_b.reshape(2, 128).transpose(1, 0)[:, :, None],
                            (128, 2, BC)).reshape(128, 128).astype(f32)),
        "b_reg": np.ascontiguousarray(np.tile(reg_b[None, :], (BC, 1)).astype(f32)),
    }

    enc = np.asarray(encoder_outputs, f32)[:, :t_enc, :]
    h0 = np.asarray(encoder_hidden, f32)[0]
    in_maps = []
    for i in range(NCORES):
        sl = slice(i * BC, (i + 1) * BC)
        enc_i = enc[sl].astype(bf16)              # [BC, t_enc, E]
        encT = (enc_i.reshape(BC, t_enc, 2, 128).transpose(3, 1, 2, 0)
                .reshape(128, t_enc * 128))
        m = dict(shared)
        m["encT"] = np.ascontiguousarray(encT)
        m["h0T"] = _feat_major(h0[sl], 4).astype(bf16)
        in_maps.append(m)
    return in_maps


def kernel(encoder_outputs, encoder_hidden, emb_W, emb_b, w_ih, w_hh,
           b_ih, b_hh, out_W, out_b, reg_W, reg_b):
    from concourse.bass_utils import run_bass_kernel_spmd

    nc = build_program()
    in_maps = prep_inputs(encoder_outputs, encoder_hidden, emb_W, emb_b,
                          w_ih, w_hh, b_ih, b_hh, out_W, out_b, reg_W, reg_b)
    res = run_bass_kernel_spmd(nc, in_maps, core_ids=list(range(NCORES)))
    out = np.empty((B, T_ALL, O), np.float32)
    for i in range(NCORES):
        out[i * BC:(i + 1) * BC] = res.results[i]["y"]
    return out
